# revision 6
# baseline (speedup 1.0000x reference)
"""Trainium2 Bass kernel for nn_ABS_MHAtt (masked two-round multi-head attention).

Strategy: pure data-parallel over batch (B=16 -> 2 batches per NeuronCore, 8 cores,
no collectives). Host-side preprocessing (inside kernel()) pre-transposes
activations/weights into the [contraction, free] layouts the TensorEngine wants and
pre-converts everything to bf16, so the device kernel does zero layout conversion.

Per-core device kernel (per batch):
  - qhT/khT projections in transposed form [o, i]; v projected in natural form [j, o]
    directly into an "augmented" layout with a ones column per head (the ones column
    makes the PV/AV matmul also produce the softmax row-sum).
  - Per head: scores computed transposed [j, i] (contraction over d=64), exp on
    ScalarE, masking by multiplying with (1-mask)^T on VectorE, PV/AV matmuls with
    the ones column, per-partition normalization with the reciprocal row-sum, and
    small PE transposes to return to [d, i] layout for the next matmul.
"""

import os
import sys

import numpy as np


def _ensure_concourse():
    try:
        import concourse.bass  # noqa: F401
        return
    except Exception:
        pass
    for p in ("/opt/trn_rl_repo", "/root/.axon_site/_ro/trn_rl_repo"):
        if os.path.isdir(p) and p not in sys.path:
            sys.path.insert(0, p)
            try:
                import concourse.bass  # noqa: F401
                return
            except Exception:
                sys.path.remove(p)
    raise ImportError("cannot import concourse (bass)")


B, L, HS = 16, 512, 1024
H, D = 16, 64
NCORES = 8
BPC = B // NCORES  # batches per core
SCALE = 1.0 / 8.0  # 1/sqrt(D)
AUGW = 65  # per-head augmented width (D + ones column)

_CACHE = {}


def _build_nc():
    _ensure_concourse()
    import concourse.bass as bass  # noqa: F401
    import concourse.mybir as mybir
    import concourse.tile as tile
    from concourse import bacc
    from contextlib import ExitStack

    bf = mybir.dt.bfloat16
    f32 = mybir.dt.float32
    Exp = mybir.ActivationFunctionType.Exp

    nc = bacc.Bacc()

    qt = nc.declare_dram_parameter("qt", [BPC, HS, L], bf, isOutput=False)
    kt = nc.declare_dram_parameter("kt", [BPC, HS, L], bf, isOutput=False)
    vt = nc.declare_dram_parameter("vt", [BPC, HS, L], bf, isOutput=False)
    imt = nc.declare_dram_parameter("imt", [BPC, HS, L], bf, isOutput=False)
    aug = nc.declare_dram_parameter("aug", [BPC, L, H * AUGW], bf, isOutput=False)
    kp1 = nc.declare_dram_parameter("kp1", [BPC, L, L], bf, isOutput=False)
    kp2 = nc.declare_dram_parameter("kp2", [BPC, L, L], bf, isOutput=False)
    wq = nc.declare_dram_parameter("wq", [HS, HS], bf, isOutput=False)
    wk = nc.declare_dram_parameter("wk", [HS, HS], bf, isOutput=False)
    wv = nc.declare_dram_parameter("wv", [HS, HS], bf, isOutput=False)
    wm = nc.declare_dram_parameter("wm", [HS, HS], bf, isOutput=False)
    idt = nc.declare_dram_parameter("idt", [128, 128], bf, isOutput=False)
    out = nc.declare_dram_parameter("out", [BPC, L, HS], f32, isOutput=True)

    with ExitStack() as ctx:
        tc = ctx.enter_context(tile.TileContext(nc))
        consts = ctx.enter_context(tc.tile_pool(name="consts", bufs=1))
        inp = ctx.enter_context(tc.tile_pool(name="inp", bufs=1))
        proj = ctx.enter_context(tc.tile_pool(name="proj", bufs=1))
        ework = ctx.enter_context(tc.tile_pool(name="ework", bufs=2))
        small = ctx.enter_context(tc.tile_pool(name="small", bufs=4))
        evac = ctx.enter_context(tc.tile_pool(name="evac", bufs=2))
        psA = ctx.enter_context(tc.tile_pool(name="psA", bufs=2, space="PSUM"))
        psPV = ctx.enter_context(tc.tile_pool(name="psPV", bufs=2, space="PSUM"))
        psTR = ctx.enter_context(tc.tile_pool(name="psTR", bufs=2, space="PSUM"))

        ident = consts.tile([128, 128], bf, tag="ident")
        nc.sync.dma_start(out=ident, in_=idt[:, :])

        w_sb = {}
        for name, wext in (("wq", wq), ("wk", wk), ("wv", wv), ("wm", wm)):
            t = consts.tile([128, 8, HS], bf, tag=name)
            nc.sync.dma_start(out=t, in_=wext.rearrange("(t p) o -> p t o", p=128))
            w_sb[name] = t

        for b in range(BPC):
            qt_sb = inp.tile([128, 8, L], bf, tag="qt")
            kt_sb = inp.tile([128, 8, L], bf, tag="kt")
            vt_sb = inp.tile([128, 8, L], bf, tag="vt")
            imt_sb = inp.tile([128, 8, L], bf, tag="imt")
            for t, ext in ((qt_sb, qt), (kt_sb, kt), (vt_sb, vt), (imt_sb, imt)):
                nc.sync.dma_start(
                    out=t, in_=ext[b].rearrange("(t p) i -> p t i", p=128)
                )
            aug_sb = inp.tile([128, 4, H * AUGW], bf, tag="aug")
            nc.sync.dma_start(
                out=aug_sb, in_=aug[b].rearrange("(t p) x -> p t x", p=128)
            )
            kp1_sb = inp.tile([128, 4, L], bf, tag="kp1")
            kp2_sb = inp.tile([128, 4, L], bf, tag="kp2")
            nc.sync.dma_start(
                out=kp1_sb, in_=kp1[b].rearrange("(t p) i -> p t i", p=128)
            )
            nc.sync.dma_start(
                out=kp2_sb, in_=kp2[b].rearrange("(t p) i -> p t i", p=128)
            )

            # ---- projections qhT = Wq @ q^T, khT = Wk @ k^T  (layout [o, i]) ----
            qh_sb = proj.tile([128, 8, L], bf, tag="qh")
            kh_sb = proj.tile([128, 8, L], bf, tag="kh")
            for wname, xsb, dst in (("wq", qt_sb, qh_sb), ("wk", kt_sb, kh_sb)):
                wt = w_sb[wname]
                for ot in range(8):
                    ps = psA.tile([128, 512], f32, tag="psA")
                    for kc in range(8):
                        nc.tensor.matmul(
                            ps,
                            wt[:, kc, ot * 128 : (ot + 1) * 128],
                            xsb[:, kc, :],
                            start=(kc == 0),
                            stop=(kc == 7),
                        )
                    nc.scalar.copy(out=dst[:, ot, :], in_=ps)

            # ---- v projection (natural [j, o]) into augmented layout + ones ----
            vaug_sb = proj.tile([128, 4, H * AUGW], bf, tag="vaug")
            for jt in range(4):
                nc.vector.memset(
                    vaug_sb[:, jt, :].rearrange("p (h x) -> p h x", x=AUGW)[:, :, 64],
                    1.0,
                )
                for oh in range(2):
                    ps = psA.tile([128, 512], f32, tag="psA")
                    for kc in range(8):
                        nc.tensor.matmul(
                            ps,
                            vt_sb[:, kc, jt * 128 : (jt + 1) * 128],
                            w_sb["wv"][:, kc, oh * 512 : (oh + 1) * 512],
                            start=(kc == 0),
                            stop=(kc == 7),
                        )
                    dst_ap = vaug_sb[
                        :, jt, oh * 8 * AUGW : (oh + 1) * 8 * AUGW
                    ].rearrange("p (h x) -> p h x", x=AUGW)[:, :, 0:64]
                    nc.scalar.copy(
                        out=dst_ap, in_=ps.rearrange("p (h x) -> p h x", x=64)
                    )

            att_sb = proj.tile([128, 8, L], bf, tag="att")

            # ---- attention, head pairs ----
            for hp in range(8):
                heads = (2 * hp, 2 * hp + 1)

                def hslice(t, h):
                    # rows h*64 .. h*64+63 of a [1024, L]-layout sbuf tensor
                    return t[(h % 2) * 64 : (h % 2) * 64 + 64, h // 2, :]

                # ---- round 1: modulate q with positional attention ----
                e1 = ework.tile([128, 4, 2, L], bf, tag="e1")
                for jt in range(4):
                    ps = psA.tile([128, 1024], f32, tag="psA")
                    for g, h in enumerate(heads):
                        nc.tensor.matmul(
                            ps[:, g * 512 : (g + 1) * 512],
                            imt_sb[
                                (h % 2) * 64 : (h % 2) * 64 + 64,
                                h // 2,
                                jt * 128 : (jt + 1) * 128,
                            ],
                            hslice(qh_sb, h),
                            start=True,
                            stop=True,
                        )
                    nc.scalar.activation(
                        out=e1[:, jt],
                        in_=ps.rearrange("p (g x) -> p g x", x=512),
                        func=Exp,
                        scale=SCALE,
                    )
                    for g in range(2):
                        nc.vector.tensor_mul(
                            e1[:, jt, g], e1[:, jt, g], kp1_sb[:, jt, :]
                        )

                # delta for both heads, stacked: dl[:, it, g*64+d]
                dl = small.tile([128, 4, 128], bf, tag="dl")
                for g, h in enumerate(heads):
                    pspv = psPV.tile([128, 4 * AUGW], f32, tag="pv")
                    for it in range(4):
                        for jt in range(4):
                            nc.tensor.matmul(
                                pspv[:, it * AUGW : it * AUGW + AUGW],
                                e1[:, jt, g, it * 128 : (it + 1) * 128],
                                aug_sb[:, jt, h * AUGW : (h + 1) * AUGW],
                                start=(jt == 0),
                                stop=(jt == 3),
                            )
                    r1 = small.tile([128, 4], f32, tag="r1")
                    nc.vector.reciprocal(
                        r1, pspv.rearrange("p (i x) -> p i x", x=AUGW)[:, :, 64]
                    )
                    for it in range(4):
                        nc.vector.tensor_scalar_mul(
                            dl[:, it, g * 64 : (g + 1) * 64],
                            pspv[:, it * AUGW : it * AUGW + 64],
                            r1[:, it : it + 1],
                        )
                # transpose both heads at once: pst[g*64+d, i] = delta_g[i, d]
                pst = psTR.tile([128, 512], bf, tag="tr")
                for it in range(4):
                    nc.tensor.transpose(
                        pst[:, it * 128 : (it + 1) * 128], dl[:, it], ident
                    )
                # qn[g*64+d, i] = qh[g*64+d, i] + delta^T  (both heads at once)
                qn_pair = small.tile([128, 512], bf, tag="qn")
                nc.vector.tensor_add(qn_pair, pst, qh_sb[:, hp, :])

                # ---- round 2: main attention ----
                e2 = ework.tile([128, 4, 2, L], bf, tag="e2")
                for jt in range(4):
                    ps = psA.tile([128, 1024], f32, tag="psA")
                    for g, h in enumerate(heads):
                        nc.tensor.matmul(
                            ps[:, g * 512 : (g + 1) * 512],
                            kh_sb[
                                (h % 2) * 64 : (h % 2) * 64 + 64,
                                h // 2,
                                jt * 128 : (jt + 1) * 128,
                            ],
                            qn_pair[(h % 2) * 64 : (h % 2) * 64 + 64, :],
                            start=True,
                            stop=True,
                        )
                    nc.scalar.activation(
                        out=e2[:, jt],
                        in_=ps.rearrange("p (g x) -> p g x", x=512),
                        func=Exp,
                        scale=SCALE,
                    )
                    for g in range(2):
                        nc.vector.tensor_mul(
                            e2[:, jt, g], e2[:, jt, g], kp2_sb[:, jt, :]
                        )

                at = small.tile([128, 4, 128], bf, tag="dl")
                for g, h in enumerate(heads):
                    psav = psPV.tile([128, 4 * AUGW], f32, tag="pv")
                    for it in range(4):
                        for jt in range(4):
                            nc.tensor.matmul(
                                psav[:, it * AUGW : it * AUGW + AUGW],
                                e2[:, jt, g, it * 128 : (it + 1) * 128],
                                vaug_sb[:, jt, h * AUGW : (h + 1) * AUGW],
                                start=(jt == 0),
                                stop=(jt == 3),
                            )
                    r2 = small.tile([128, 4], f32, tag="r1")
                    nc.vector.reciprocal(
                        r2, psav.rearrange("p (i x) -> p i x", x=AUGW)[:, :, 64]
                    )
                    for it in range(4):
                        nc.vector.tensor_scalar_mul(
                            at[:, it, g * 64 : (g + 1) * 64],
                            psav[:, it * AUGW : it * AUGW + 64],
                            r2[:, it : it + 1],
                        )
                pst2 = psTR.tile([128, 512], bf, tag="tr")
                for it in range(4):
                    nc.tensor.transpose(
                        pst2[:, it * 128 : (it + 1) * 128], at[:, it], ident
                    )
                nc.scalar.copy(out=att_sb[:, hp, :], in_=pst2)

            # ---- output projection: out[i, o] = attT^T @ WmT ----
            for it in range(4):
                for oh in range(2):
                    ps = psA.tile([128, 512], f32, tag="psA")
                    for kc in range(8):
                        nc.tensor.matmul(
                            ps,
                            att_sb[:, kc, it * 128 : (it + 1) * 128],
                            w_sb["wm"][:, kc, oh * 512 : (oh + 1) * 512],
                            start=(kc == 0),
                            stop=(kc == 7),
                        )
                    ob = evac.tile([128, 512], f32, tag="ob")
                    nc.vector.tensor_copy(out=ob, in_=ps)
                    nc.sync.dma_start(
                        out=out[b, it * 128 : (it + 1) * 128, oh * 512 : (oh + 1) * 512],
                        in_=ob,
                    )

    nc.compile()
    return nc


def _get_nc():
    if "nc" not in _CACHE:
        _CACHE["nc"] = _build_nc()
    return _CACHE["nc"]


def _prep_inputs(v, k, q, img_abs, Wv, Wk, Wq, Wm, abs_mask, mask):
    import ml_dtypes

    bf16 = ml_dtypes.bfloat16
    f32 = np.float32

    def t_bf(x):  # [B, L, HS] -> [B, HS, L] bf16
        return np.ascontiguousarray(np.swapaxes(np.asarray(x, f32), 1, 2)).astype(bf16)

    qt = t_bf(q)
    ktr = t_bf(k)
    vtr = t_bf(v)
    imt = t_bf(img_abs)

    img = np.asarray(img_abs, f32)
    augf = np.empty((B, L, H * AUGW), f32)
    augf.reshape(B, L, H, AUGW)[..., :64] = img.reshape(B, L, H, 64)
    augf.reshape(B, L, H, AUGW)[..., 64] = 1.0
    augv = augf.astype(bf16)

    def keepT(m):  # [B, 1, L, L] bool -> (1-m)^T bf16
        kf = 1.0 - np.asarray(m, f32)[:, 0]
        return np.ascontiguousarray(np.swapaxes(kf, 1, 2)).astype(bf16)

    kp1 = keepT(abs_mask)
    kp2 = keepT(mask)

    def wT(w):
        return np.ascontiguousarray(np.asarray(w, f32).T).astype(bf16)

    wqs, wks, wvs, wms = wT(Wq), wT(Wk), wT(Wv), wT(Wm)
    ident = np.eye(128, dtype=bf16)

    in_maps = []
    for c in range(NCORES):
        s = slice(c * BPC, (c + 1) * BPC)
        in_maps.append(
            {
                "qt": qt[s],
                "kt": ktr[s],
                "vt": vtr[s],
                "imt": imt[s],
                "aug": augv[s],
                "kp1": kp1[s],
                "kp2": kp2[s],
                "wq": wqs,
                "wk": wks,
                "wv": wvs,
                "wm": wms,
                "idt": ident,
            }
        )
    return in_maps


def kernel(v, k, q, img_abs, Wv, Wk, Wq, Wm, abs_mask, mask, _trace=False):
    _ensure_concourse()
    from concourse.bass_utils import run_bass_kernel_spmd

    in_maps = _prep_inputs(v, k, q, img_abs, Wv, Wk, Wq, Wm, abs_mask, mask)
    nc = _get_nc()
    res = run_bass_kernel_spmd(nc, in_maps, core_ids=list(range(NCORES)), trace=_trace)
    outp = np.concatenate([res.results[i]["out"] for i in range(NCORES)], axis=0)
    outp = np.asarray(outp, np.float32)
    if _trace:
        _CACHE["last_result"] = res
    return outp
